# revision 17
# baseline (speedup 1.0000x reference)
"""SVGD actor sampler (nn_ActorSvgd) — 8-core data-parallel Trainium2 Bass kernel.

Sharding: batch axis B=4096 split 8 ways -> 512 batch elements (16384 rows)
per core.  Each core runs the full 3-step SVGD loop on its shard; outputs are
concatenated on host.  No collectives needed.

On-chip layouts per chunk of 128 batch elements (4096 rows):
  - obsT [64, 4096], aT [16, 4096]  (feature-major for PE matmuls)
  - PQ   [128, 32*33]: strip layout, partition p=(g,j), free (q, [X(16) S(16) t(1)])
    for batch element b = 4q+g, particle j.
  - layout A: partition p = 32g+q (= batch element b=4q+g), free (i,j) 32x32
    for dist/median/K (per-partition scalars = per-batch-element).
  - strip<->A transforms ride a DRAM round-trip with affine access patterns.
"""

import os
import sys

for _p in ("/opt/trn_rl_repo", "/root/.axon_site/_ro/trn_rl_repo"):
    if os.path.isdir(_p) and _p not in sys.path:
        sys.path.insert(0, _p)

import numpy as np

import concourse.bass as bass
import concourse.bacc as bacc_mod
import concourse.mybir as mybir
from concourse.tile import TileContext
from concourse.bass_utils import run_bass_kernel_spmd

FP = mybir.dt.float32
FPR = mybir.dt.float32r
AX = mybir.AxisListType
AL = mybir.AluOpType
AF = mybir.ActivationFunctionType

B, N, D, DOBS, H = 4096, 32, 16, 64, 256
NCORE = 8
BL = B // NCORE        # 512 batch elems / core
NCH = 4                # chunks / core
CB = 128               # batch elems / chunk
RC = CB * N            # 4096 rows / chunk
NCOLT = RC // 512      # 8 column tiles of 512 rows
STEPS, LR, LIMIT = 3, 0.1, 1.0
LN_N = float(np.log(N))
BIGF = 1.0e30
MED_ITERS = 18

_CACHE = {}


def _build():
    nc = bacc_mod.Bacc()
    obs_dr = nc.declare_dram_parameter("obs", [BL * N, DOBS], FP, isOutput=False)
    a_dr = nc.declare_dram_parameter("a", [BL * N, D], FP, isOutput=False)
    w1_dr = nc.declare_dram_parameter("w1", [DOBS + D, H], FP, isOutput=False)
    b1_dr = nc.declare_dram_parameter("b1", [H], FP, isOutput=False)
    w2_dr = nc.declare_dram_parameter("w2", [H, H], FP, isOutput=False)
    b2_dr = nc.declare_dram_parameter("b2", [H], FP, isOutput=False)
    w3_dr = nc.declare_dram_parameter("w3", [H, 1], FP, isOutput=False)
    b3_dr = nc.declare_dram_parameter("b3", [1], FP, isOutput=False)
    aout_dr = nc.declare_dram_parameter("a_out", [BL * N, D], FP, isOutput=True)
    logp_dr = nc.declare_dram_parameter("logp", [BL, N], FP, isOutput=True)
    qv_dr = nc.declare_dram_parameter("q_vals", [BL * N], FP, isOutput=True)

    def r32(ap):
        return ap

    with TileContext(nc) as tc:
        with (
            tc.tile_pool(name="const", bufs=1) as cpool,
            tc.tile_pool(name="wts", bufs=1) as wpool,
            tc.tile_pool(name="chunk", bufs=1) as chpool,
            tc.tile_pool(name="mlp", bufs=2) as mpool,
            tc.tile_pool(name="sv", bufs=1) as svpool,
            tc.tile_pool(name="small", bufs=1) as smpool,
            tc.tile_pool(name="ps512", bufs=1, space="PSUM") as pp512,
            tc.tile_pool(name="psmisc", bufs=1, space="PSUM") as ppm,
            tc.tile_pool(name="pstr", bufs=2, space="PSUM") as pptr,
            tc.tile_pool(name="dram", bufs=2, space="DRAM") as dpool,
        ):
            # ---- constants / weights (once) ----
            ident = cpool.tile([128, 128], FP, tag="ident")
            nc.vector.memset(ident[:], 1.0)
            nc.gpsimd.affine_select(
                ident[:], ident[:], pattern=[[1, 128]], compare_op=AL.is_equal,
                fill=0.0, base=0, channel_multiplier=-1,
            )
            ones32 = cpool.tile([128, 32], FP, tag="ones32")
            nc.vector.memset(ones32[:], 1.0)

            w1sb = wpool.tile([DOBS + D, H], FP, tag="w1sb")
            nc.gpsimd.dma_start(w1sb[:], w1_dr[:])
            w1asb = wpool.tile([D, H], FP, tag="w1asb")
            nc.gpsimd.dma_start(w1asb[:], w1_dr[DOBS:DOBS + D, :])
            w2sb = wpool.tile([128, 512], FP, tag="w2sb")
            nc.gpsimd.dma_start(
                w2sb[:].rearrange("p (k m) -> p k m", m=256),
                w2_dr[:].rearrange("(k p) m -> p k m", p=128),
            )
            w3sb = wpool.tile([128, 2], FP, tag="w3sb")
            nc.gpsimd.dma_start(
                w3sb[:].rearrange("p (k one) -> p k one", one=1),
                w3_dr[:].rearrange("(k p) one -> p k one", p=128),
            )
            b1sb = wpool.tile([128, 2], FP, tag="b1sb")
            nc.gpsimd.dma_start(
                b1sb[:].rearrange("p (k one) -> p k one", one=1),
                b1_dr[:].rearrange("(k p one) -> p k one", p=128, one=1),
            )
            b2sb = wpool.tile([128, 2], FP, tag="b2sb")
            nc.gpsimd.dma_start(
                b2sb[:].rearrange("p (k one) -> p k one", one=1),
                b2_dr[:].rearrange("(k p one) -> p k one", p=128, one=1),
            )
            b3sb = wpool.tile([1, 1], FP, tag="b3sb")
            nc.gpsimd.dma_start(b3sb[:], b3_dr[:].rearrange("(one o2) -> one o2", one=1))
            b3bc = wpool.tile([128, 1], FP, tag="b3bc")
            nc.gpsimd.partition_broadcast(b3bc[:], b3sb[:])

            # compute-engine copies: matmul weight-loads have a tiny sync-wait
            # table, so never let a matmul consume a DMA output directly.
            w1c = wpool.tile([DOBS + D, H], FP, tag="w1c")
            nc.scalar.copy(w1c[:], w1sb[:])
            w1ac = wpool.tile([D, H], FP, tag="w1ac")
            nc.scalar.copy(w1ac[:], w1asb[:])
            w2c = wpool.tile([128, 512], FP, tag="w2c")
            nc.scalar.copy(w2c[:], w2sb[:])
            w3c = wpool.tile([128, 2], FP, tag="w3c")
            nc.scalar.copy(w3c[:], w3sb[:])
            # w2T chunks: w2tsb[p, m*256 + k*128 + c] = w2[k*128+c, m*128+p]
            w2tsb = wpool.tile([128, 512], FP, tag="w2tsb")
            for k in range(2):
                for m in range(2):
                    pst = pptr.tile([128, 128], FP, tag="ptr")
                    nc.tensor.matmul(
                        pst[:], w2c[:, k * 256 + m * 128:k * 256 + (m + 1) * 128],
                        ident[:], start=True, stop=True,
                    )
                    nc.scalar.copy(
                        w2tsb[:, m * 256 + k * 128:m * 256 + (k + 1) * 128], pst[:]
                    )
            # w1aT[p, k*16+d] = w1[64+d, k*128+p]
            w1aT = wpool.tile([128, 32], FP, tag="w1aT")
            for k in range(2):
                pst = pptr.tile([128, 128], FP, tag="ptr")
                nc.tensor.matmul(
                    pst[0:128, 0:16],
                    w1ac[:, k * 128:(k + 1) * 128],
                    ident[0:16, 0:16], start=True, stop=True,
                )
                nc.scalar.copy(w1aT[:, k * 16:(k + 1) * 16], pst[0:128, 0:16])

            # ---- per-chunk processing ----
            for c in range(NCH):
                obst_raw = chpool.tile([128, 32 * DOBS], FP, tag="obst_raw")
                nc.gpsimd.dma_start(
                    obst_raw[:].rearrange("p (t d) -> p t d", d=DOBS),
                    obs_dr[:].rearrange("(cc t p) d -> cc p t d", t=32, p=128)[c],
                )
                obst_c = chpool.tile([128, 32 * DOBS], FP, tag="obst_c")
                nc.scalar.copy(obst_c[:], obst_raw[:])
                obsT = chpool.tile([DOBS, RC], FP, tag="obsT")
                for t4 in range(8):
                    psot = pptr.tile([64, 512], FP, tag="ptr")
                    for t in range(4):
                        nc.tensor.matmul(
                            psot[:, t * 128:(t + 1) * 128],
                            obst_c[:, (t4 * 4 + t) * DOBS:(t4 * 4 + t + 1) * DOBS],
                            ident[:], start=True, stop=True,
                        )
                    nc.scalar.copy(obsT[:, t4 * 512:(t4 + 1) * 512], psot[:])

                PQ = chpool.tile([128, 32 * 33], FP, tag="PQ")
                pqv = PQ[:].rearrange("p (q e) -> p q e", e=33)
                a_raw = chpool.tile([128, 512], FP, tag="a_raw")
                nc.gpsimd.dma_start(
                    a_raw[:].rearrange("p (q d) -> p q d", d=D),
                    a_dr[:].rearrange("(cc q p) d -> cc p q d", q=32, p=128)[c],
                )
                nc.scalar.copy(pqv[:, :, 0:D], a_raw[:].rearrange("p (q d) -> p q d", d=D))
                logpA = chpool.tile([128, 32], FP, tag="logpA")
                nc.vector.memset(logpA[:], 0.0)
                qv = chpool.tile([128, 32], FP, tag="qv")

                for s in range(STEPS):
                    # ---- aT from PQ X-columns ----
                    aT = chpool.tile([D, RC], FP, tag="aT")
                    for q4 in range(8):
                        psat = pptr.tile([16, 512], FP, tag="ptr")
                        for qq in range(4):
                            nc.tensor.matmul(
                                psat[:, qq * 128:(qq + 1) * 128],
                                pqv[:, q4 * 4 + qq, 0:D],
                                ident[:], start=True, stop=True,
                            )
                        nc.scalar.copy(aT[:, q4 * 512:(q4 + 1) * 512], psat[:])

                    # ---- MLP forward + backward per column tile ----
                    for ct in range(NCOLT):
                        cs = slice(ct * 512, (ct + 1) * 512)
                        h1 = mpool.tile([128, 1024], FP, tag="h1")
                        for m in range(2):
                            ps1 = pp512.tile([128, 512], FP, tag="ps1")
                            nc.tensor.matmul(
                                ps1[:], r32(w1c[0:DOBS, m * 128:(m + 1) * 128]),
                                r32(obsT[:, cs]), start=True, stop=False,
                            )
                            nc.tensor.matmul(
                                ps1[:],
                                r32(w1ac[:, m * 128:(m + 1) * 128]),
                                r32(aT[:, cs]), start=False, stop=True,
                            )
                            nc.scalar.activation(
                                h1[:, m * 512:(m + 1) * 512], ps1[:], AF.Relu,
                                bias=b1sb[:, m:m + 1], scale=1.0,
                            )
                        g2 = mpool.tile([128, 1024], FP, tag="g2")
                        if s == 2:
                            h2 = mpool.tile([128, 1024], FP, tag="h2")
                        else:
                            h2 = None
                        for m in range(2):
                            ps2 = pp512.tile([128, 512], FP, tag="ps2")
                            for k in range(2):
                                nc.tensor.matmul(
                                    ps2[:],
                                    r32(w2c[:, k * 256 + m * 128:k * 256 + (m + 1) * 128]),
                                    r32(h1[:, k * 512:(k + 1) * 512]),
                                    start=(k == 0), stop=(k == 1),
                                )
                            if s == 2:
                                nc.scalar.activation(
                                    h2[:, m * 512:(m + 1) * 512], ps2[:], AF.Relu,
                                    bias=b2sb[:, m:m + 1], scale=1.0,
                                )
                            # g2 = (pre2 > 0) * w3[f]   (b2 == bias add first)
                            nc.vector.tensor_scalar(
                                g2[:, m * 512:(m + 1) * 512], ps2[:],
                                b2sb[:, m:m + 1], 0.0, AL.add, AL.bypass,
                            )
                            nc.vector.tensor_scalar(
                                g2[:, m * 512:(m + 1) * 512],
                                g2[:, m * 512:(m + 1) * 512],
                                0.0, w3sb[:, m:m + 1], AL.is_gt, AL.mult,
                            )
                        if s == 2:
                            psq = ppm.tile([128, 4], FP, tag="psqs")
                            for rc4 in range(4):
                                for k in range(2):
                                    nc.tensor.matmul(
                                        psq[:, rc4:rc4 + 1],
                                        r32(h2[:, k * 512 + rc4 * 128:k * 512 + (rc4 + 1) * 128]),
                                        r32(w3c[:, k:k + 1]),
                                        start=(k == 0), stop=(k == 1),
                                    )
                            nc.scalar.activation(
                                qv[:, ct * 4:(ct + 1) * 4], psq[:], AF.Identity,
                                bias=b3bc[:, 0:1], scale=1.0,
                            )
                        g1 = mpool.tile([128, 1024], FP, tag="g1")
                        msk = mpool.tile([128, 1024], FP, tag="msk")
                        nc.vector.tensor_scalar(
                            msk[:], h1[:], 0.0, None, AL.is_gt
                        )
                        for m in range(2):
                            psg = pp512.tile([128, 512], FP, tag="psg")
                            for k in range(2):
                                nc.tensor.matmul(
                                    psg[:],
                                    r32(w2tsb[:, m * 256 + k * 128:m * 256 + (k + 1) * 128]),
                                    r32(g2[:, k * 512:(k + 1) * 512]),
                                    start=(k == 0), stop=(k == 1),
                                )
                            nc.vector.tensor_tensor(
                                g1[:, m * 512:(m + 1) * 512],
                                msk[:, m * 512:(m + 1) * 512], psg[:], AL.mult,
                            )
                        pss = ppm.tile([128, 64], FP, tag="psqs")
                        for rc4 in range(4):
                            for k in range(2):
                                nc.tensor.matmul(
                                    pss[:, rc4 * 16:(rc4 + 1) * 16],
                                    r32(g1[:, k * 512 + rc4 * 128:k * 512 + (rc4 + 1) * 128]),
                                    r32(w1aT[:, k * 16:(k + 1) * 16]),
                                    start=(k == 0), stop=(k == 1),
                                )
                        nc.vector.tensor_copy(
                            pqv[:, ct * 4:(ct + 1) * 4, D:2 * D],
                            pss[:].rearrange("p (r d) -> p r d", d=16),
                        )

                    # ---- t_j = X.S  and r_j = X.X ----
                    tmp1t = svpool.tile([128, 512], FP, tag="tmp1t")
                    nc.vector.tensor_tensor(
                        tmp1t[:].rearrange("p (q d) -> p q d", d=16),
                        pqv[:, :, 0:D], pqv[:, :, D:2 * D], AL.mult,
                    )
                    nc.vector.reduce_sum(
                        pqv[:, :, 2 * D:2 * D + 1], tmp1t[:].rearrange("p (q d) -> p q d", d=16),
                        axis=AX.X,
                    )
                    nc.vector.tensor_tensor(
                        tmp1t[:].rearrange("p (q d) -> p q d", d=16),
                        pqv[:, :, 0:D], pqv[:, :, 0:D], AL.mult,
                    )
                    rstrip = smpool.tile([128, 32], FP, tag="rstrip")
                    nc.vector.reduce_sum(
                        rstrip[:], tmp1t[:].rearrange("p (q d) -> p q d", d=16),
                        axis=AX.X,
                    )
                    rA = smpool.tile([128, 32], FP, tag="rA")
                    nc.vector.transpose(rA[:], rstrip[:])

                    # ---- Gram via PE (strip layout), round-trip to layout A ----
                    GB = svpool.tile([128, 1024], FP, tag="GB")
                    for q8 in range(4):
                        psG = ppm.tile([128, 256], FP, tag="psG")
                        for q in range(8):
                            qq = q8 * 8 + q
                            for g in range(4):
                                bb = qq * 4 + g
                                nc.tensor.matmul(
                                    psG[32 * g:32 * (g + 1), q * 32:(q + 1) * 32],
                                    r32(aT[:, bb * 32:(bb + 1) * 32]),
                                    r32(aT[:, bb * 32:(bb + 1) * 32]),
                                    start=True, stop=True,
                                    tile_position=(0, 32 * g),
                                )
                        nc.vector.tensor_copy(
                            GB[:, q8 * 256:(q8 + 1) * 256], psG[:]
                        )
                    gdr = dpool.tile([CB, 32, 32], FP, tag="gdr")
                    for g in range(4):
                        nc.gpsimd.dma_start(
                            gdr[:].rearrange("(q gg) i j -> gg i q j", gg=4)[g],
                            GB[32 * g:32 * (g + 1), :].rearrange("p (q j) -> p q j", j=32),
                        )
                    GA = svpool.tile([128, 1024], FP, tag="GA")
                    for g in range(4):
                        nc.gpsimd.dma_start(
                            GA[32 * g:32 * (g + 1), :].rearrange("p (i j) -> p i j", j=32),
                            gdr[:].rearrange("(q gg) i j -> gg q i j", gg=4)[g],
                        )

                    # ---- dist = r_i + r_j - 2G (layout A) ----
                    dist = svpool.tile([128, 1024], FP, tag="dist")
                    dv = dist[:].rearrange("p (i j) -> p i j", j=32)
                    nc.vector.tensor_tensor(
                        dv,
                        rA[:].rearrange("p (i o) -> p i o", o=1).broadcast_to([128, 32, 32]),
                        rA[:].rearrange("p (o j) -> p o j", o=1).broadcast_to([128, 32, 32]),
                        AL.add,
                    )
                    G2t = svpool.tile([128, 1024], FP, tag="G2t")
                    nc.vector.tensor_scalar(G2t[:], GA[:], 2.0, None, AL.mult)
                    nc.vector.tensor_tensor(dist[:], dist[:], G2t[:], AL.subtract)

                    # ---- median via bisection + exact-gap extraction ----
                    lo = smpool.tile([128, 1], FP, tag="lo")
                    hi = smpool.tile([128, 1], FP, tag="hi")
                    mm_ = smpool.tile([128, 1], FP, tag="mm_")
                    cnt = smpool.tile([128, 1], FP, tag="cnt")
                    pm = smpool.tile([128, 1], mybir.dt.int32, tag="pm")
                    junk = svpool.tile([128, 1024], FP, tag="junk")
                    nc.vector.memset(lo[:], 0.0)
                    nc.vector.reduce_max(hi[:], dist[:], axis=AX.X)
                    for _ in range(MED_ITERS):
                        nc.vector.tensor_scalar(
                            mm_[:], lo[:], hi[:, 0:1], 0.5, AL.add, AL.mult
                        )
                        nc.vector.tensor_scalar(
                            junk[:], dist[:], mm_[:, 0:1], 0.0, AL.is_le,
                            AL.add, accum_out=cnt[:],
                        )
                        nc.vector.tensor_scalar(pm[:], cnt[:], 512.0, None, AL.is_ge)
                        nc.vector.copy_predicated(hi[:], pm[:], mm_[:])
                        nc.vector.tensor_scalar(pm[:], cnt[:], 512.0, None, AL.is_lt)
                        nc.vector.copy_predicated(lo[:], pm[:], mm_[:])
                    # s512 = min{v > hi};  med = (hi + s512)/2
                    nc.vector.tensor_scalar(
                        junk[:], dist[:], hi[:, 0:1], None, AL.is_gt
                    )
                    nc.vector.tensor_scalar(
                        junk[:], junk[:], -BIGF, BIGF, AL.mult, AL.add
                    )
                    nc.vector.tensor_tensor(junk[:], junk[:], dist[:], AL.add)
                    s512 = smpool.tile([128, 1], FP, tag="s512")
                    nc.vector.tensor_reduce(s512[:], junk[:], axis=AX.X, op=AL.min)
                    med = smpool.tile([128, 1], FP, tag="med")
                    nc.vector.tensor_scalar(
                        med[:], hi[:], s512[:, 0:1], 0.5, AL.add, AL.mult
                    )
                    gam = smpool.tile([128, 1], FP, tag="gam")
                    nc.vector.tensor_scalar(
                        gam[:], med[:], 2.0 / LN_N, 2.0e-8, AL.mult, AL.add
                    )
                    nc.vector.reciprocal(gam[:], gam[:])
                    ngam = smpool.tile([128, 1], FP, tag="ngam")
                    nc.vector.tensor_scalar(ngam[:], gam[:], -1.0, None, AL.mult)

                    # ---- K = exp(-gamma*dist); deg, w ----
                    KA = svpool.tile([128, 1024], FP, tag="KA")
                    nc.scalar.activation(
                        KA[:], dist[:], AF.Exp, bias=0.0, scale=ngam[:, 0:1]
                    )
                    degA = smpool.tile([128, 32], FP, tag="degA")
                    nc.vector.reduce_sum(
                        degA[:], KA[:].rearrange("p (i j) -> p i j", j=32), axis=AX.X
                    )
                    nc.vector.tensor_tensor(junk[:], KA[:], dist[:], AL.mult)
                    wA = smpool.tile([128, 32], FP, tag="wA")
                    nc.vector.reduce_sum(
                        wA[:], junk[:].rearrange("p (i j) -> p i j", j=32), axis=AX.X
                    )

                    # ---- K to strip layout via DRAM round trip ----
                    kdr = dpool.tile([CB, 32, 32], FP, tag="kdr")
                    for g in range(4):
                        nc.gpsimd.dma_start(
                            kdr[:].rearrange("(q gg) x y -> gg q x y", gg=4)[g],
                            KA[32 * g:32 * (g + 1), :].rearrange("p (x y) -> p x y", y=32),
                        )
                    KBr = svpool.tile([128, 1024], FP, tag="KBr")
                    for g in range(4):
                        nc.gpsimd.dma_start(
                            KBr[32 * g:32 * (g + 1), :].rearrange("p (q i) -> p q i", i=32),
                            kdr[:].rearrange("(q gg) x y -> gg x q y", gg=4)[g],
                        )
                    KB = svpool.tile([128, 1024], FP, tag="KB")
                    nc.scalar.copy(KB[:], KBr[:])

                    # ---- strip-form per-b vectors ----
                    degS = smpool.tile([128, 32], FP, tag="degS")
                    nc.vector.transpose(degS[:], degA[:])
                    wS = smpool.tile([128, 32], FP, tag="wS")
                    nc.vector.transpose(wS[:], wA[:])
                    gbc = smpool.tile([128, 32], FP, tag="gbc")
                    nc.scalar.mul(gbc[:], ones32[:], gam[:, 0:1])
                    gS = smpool.tile([128, 32], FP, tag="gS")
                    nc.vector.transpose(gS[:], gbc[:])

                    # ---- drive = K @ [X S t] per batch element ----
                    DR = svpool.tile([128, 32 * 33], FP, tag="DR")
                    drv = DR[:].rearrange("p (q e) -> p q e", e=33)
                    for q8 in range(4):
                        psD = ppm.tile([128, 264], FP, tag="psD")
                        for q in range(8):
                            qq = q8 * 8 + q
                            for g in range(4):
                                nc.tensor.matmul(
                                    psD[32 * g:32 * (g + 1), q * 33:(q + 1) * 33],
                                    r32(KB[32 * g:32 * (g + 1), qq * 32:(qq + 1) * 32]),
                                    r32(PQ[32 * g:32 * (g + 1), qq * 33:(qq + 1) * 33]),
                                    start=True, stop=True,
                                    tile_position=(32 * g, 32 * g),
                                )
                        nc.vector.tensor_copy(
                            DR[:, q8 * 264:(q8 + 1) * 264], psD[:]
                        )

                    # ---- logp terms ----
                    nc.vector.tensor_tensor(
                        tmp1t[:].rearrange("p (q d) -> p q d", d=16),
                        pqv[:, :, 0:D], drv[:, :, D:2 * D], AL.mult,
                    )
                    xdr = smpool.tile([128, 32], FP, tag="xdr")
                    nc.vector.reduce_sum(
                        xdr[:], tmp1t[:].rearrange("p (q d) -> p q d", d=16), axis=AX.X
                    )
                    c1S = smpool.tile([128, 32], FP, tag="c1S")
                    nc.vector.tensor_scalar(c1S[:], gS[:], -2.0 / N, None, AL.mult)
                    t1 = smpool.tile([128, 32], FP, tag="t1")
                    nc.vector.tensor_tensor(
                        t1[:], xdr[:], drv[:, :, 2 * D:2 * D + 1].rearrange("p q o -> p (q o)"),
                        AL.subtract,
                    )
                    nc.vector.tensor_tensor(t1[:], t1[:], c1S[:], AL.mult)
                    u1 = smpool.tile([128, 32], FP, tag="u1")
                    nc.vector.tensor_tensor(u1[:], wS[:], gS[:], AL.mult)
                    nc.vector.tensor_scalar(u1[:], u1[:], 2.0, None, AL.mult)
                    u3 = smpool.tile([128, 32], FP, tag="u3")
                    nc.vector.tensor_scalar(u3[:], degS[:], float(D), None, AL.mult)
                    nc.vector.tensor_tensor(u1[:], u1[:], u3[:], AL.subtract)
                    nc.vector.tensor_tensor(u1[:], u1[:], c1S[:], AL.mult)
                    nc.vector.tensor_tensor(t1[:], t1[:], u1[:], AL.add)
                    nc.vector.tensor_scalar(t1[:], t1[:], LR, None, AL.mult)
                    tA = smpool.tile([128, 32], FP, tag="tA")
                    nc.vector.transpose(tA[:], t1[:])
                    nc.vector.tensor_tensor(logpA[:], logpA[:], tA[:], AL.subtract)

                    # ---- phi and a update ----
                    m1 = svpool.tile([128, 512], FP, tag="m1")
                    m1v = m1[:].rearrange("p (q d) -> p q d", d=16)
                    nc.vector.tensor_tensor(
                        m1v, pqv[:, :, 0:D],
                        degS[:].rearrange("p (q o) -> p q o", o=1).broadcast_to([128, 32, 16]),
                        AL.mult,
                    )
                    nc.vector.tensor_tensor(m1v, m1v, drv[:, :, 0:D], AL.subtract)
                    nc.vector.tensor_tensor(
                        m1v, m1v,
                        gS[:].rearrange("p (q o) -> p q o", o=1).broadcast_to([128, 32, 16]),
                        AL.mult,
                    )
                    nc.vector.tensor_scalar(
                        m1[:], m1[:], 2.0 * LR / N, None, AL.mult
                    )
                    s1t = svpool.tile([128, 512], FP, tag="s1t")
                    s1v = s1t[:].rearrange("p (q d) -> p q d", d=16)
                    nc.vector.tensor_scalar(
                        s1v, drv[:, :, D:2 * D], LR / N, None, AL.mult
                    )
                    nc.vector.tensor_tensor(m1[:], m1[:], s1t[:], AL.add)
                    nc.vector.tensor_tensor(m1v, m1v, pqv[:, :, 0:D], AL.add)
                    nc.vector.tensor_scalar(
                        pqv[:, :, 0:D], m1v, LIMIT, -LIMIT, AL.min, AL.max
                    )

                # ---- chunk outputs ----
                for g in range(4):
                    nc.gpsimd.dma_start(
                        aout_dr[:].rearrange(
                            "(cc q gg j) d -> cc gg j q d", q=32, gg=4, j=32
                        )[c, g],
                        PQ[32 * g:32 * (g + 1), :].rearrange("p (q e) -> p q e", e=33)[:, :, 0:D],
                    )
                    nc.gpsimd.dma_start(
                        logp_dr[:].rearrange("(cc q gg) i -> cc gg q i", q=32, gg=4)[c, g],
                        logpA[32 * g:32 * (g + 1), :],
                    )
                qvT = chpool.tile([128, 32], FP, tag="qvT")
                nc.vector.transpose(qvT[:], qv[:])
                for pb in range(4):
                    nc.gpsimd.dma_start(
                        qv_dr[:].rearrange(
                            "(cc cols pb j) -> cc pb cols j", cols=32, pb=4, j=32
                        )[c, pb],
                        qvT[32 * pb:32 * (pb + 1), :],
                    )
    nc.finalize()
    return nc


def kernel(**inputs):
    if "nc" not in _CACHE:
        _CACHE["nc"] = _build()
    nc = _CACHE["nc"]
    obs = np.ascontiguousarray(inputs["obs"], dtype=np.float32)
    a = np.ascontiguousarray(inputs["a"], dtype=np.float32)
    wk = {
        k: np.ascontiguousarray(inputs[k], dtype=np.float32)
        for k in ("w1", "b1", "w2", "b2", "w3", "b3")
    }
    in_maps = []
    for core in range(NCORE):
        rs = slice(core * BL * N, (core + 1) * BL * N)
        im = {"obs": obs[rs], "a": a[rs]}
        im.update(wk)
        in_maps.append(im)
    res = run_bass_kernel_spmd(nc, in_maps, core_ids=list(range(NCORE)))
    outs = res.results
    a_out = np.concatenate([o["a_out"] for o in outs], axis=0).reshape(B, N, D)
    logp = np.concatenate([o["logp"] for o in outs], axis=0).reshape(B, N)
    q_vals = np.concatenate([o["q_vals"] for o in outs], axis=0).reshape(B * N)
    return a_out, logp, q_vals
